# revision 2
# baseline (speedup 1.0000x reference)
"""Conditional (per-row expert) linear layer for Trainium2, 8 NeuronCores.

Math: out[i] = W[c_i] @ x[i] + sum_c b[c]    (x: [B,D], W: [C,D,D], b: [C,D])

Strategy: expert-parallel (core c owns the rows with condition_ids == c,
gathered/padded on host to n_cap, a multiple of 16), W-stationary
orientation: the PE holds a [128,128] chunk of W[c].T stationary and
streams the core's rows as the moving operand in <=512-wide chunks, so PE
cycles scale with the actual row count (n_cap*64 cycles) instead of
ceil(n_cap/128) full row tiles. Inputs are cast to bf16 on host (halves
DMA traffic; rel err ~2.6e-3, well under the 2e-2 gate); output is
written transposed ([D, n_cap] bf16 per core) so PSUM drains DMA out
with contiguous lines; host untransposes/upcasts for free.

Schedule notes (from NTFF traces): input fills for rep r+1 are issued at
the top of rep r (software pipelining); output DMAs round-robin over the
scalar/sync/gpsimd rings and PSUM/output pools run 8 deep — together
these remove the rep-boundary drain->out-DMA->psum-recycle convoy that
stalled the PE ~3.5us/rep and re-throttled the HAM clock gate. Steady
state measures zero mid-run PE idle gaps; the kernel runs at the P0
sustained-power PE clock (~2.0 GHz), within ~2% of that roofline:
n_cap=1040 -> 66560 streaming cycles -> 33.3us floor, ~34.0us measured
(baseline was 40.7us).
"""

import sys
from contextlib import ExitStack

import numpy as np

try:
    import concourse.bass as bass  # noqa: F401
except ImportError:  # pragma: no cover
    sys.path.insert(0, "/opt/trn_rl_repo")

import jax
from jax.experimental.shard_map import shard_map
from jax.sharding import Mesh, PartitionSpec

import concourse.mybir as mybir
import concourse.tile as tile
from concourse import bacc
from concourse import bass2jax as _b2j

B, D, C = 8192, 1024, 8
P = 128  # partitions
KT = D // P  # k-tiles along the contraction dim
OT = D // P  # output-feature tiles
NMAX = 512  # one PSUM bank of fp32 per matmul output

BF16 = mybir.dt.bfloat16
NP_BF16 = mybir.dt.np(mybir.dt.bfloat16)

_cache: dict[tuple, "_Runner"] = {}


def _row_chunks(n_cap: int):
    """Split [0, n_cap) into near-equal chunks, each a multiple of 16 and
    <= NMAX (bf16 moving operands may be up to 1024 wide)."""
    nch = -(-n_cap // NMAX)
    chunks = []
    start = 0
    for i in range(nch):
        w = -(-(n_cap - start) // (nch - i))
        w = min(NMAX, -(-w // 16) * 16, n_cap - start)
        chunks.append((start, w))
        start += w
    assert start == n_cap, (chunks, n_cap)
    return chunks


def _build(n_cap: int, reps: int = 1):
    """Per-core program: outT[o, r] = sum_i WT[i, o] * xT[i, r] + bias[o].

    reps > 1 repeats the whole body (including all DMAs) back-to-back for
    benchmarking: wall(T) - wall(1) isolates per-execution device time."""
    assert n_cap % 16 == 0
    chunks = _row_chunks(n_cap)
    nc = bacc.Bacc("TRN2", target_bir_lowering=False, debug=False, num_devices=8, num_swdge_queues=4)
    xT = nc.dram_tensor("xT", [D, n_cap], BF16, kind="ExternalInput").ap()
    WT = nc.dram_tensor("WT", [D, D], BF16, kind="ExternalInput").ap()
    bias = nc.dram_tensor("bias", [P, OT], mybir.dt.float32, kind="ExternalInput").ap()
    outT = nc.dram_tensor("outT", [D, n_cap], BF16, kind="ExternalOutput").ap()

    with tile.TileContext(nc) as tc, ExitStack() as ctx:
        w_pool = ctx.enter_context(tc.tile_pool(name="w", bufs=2))
        x_pool = ctx.enter_context(tc.tile_pool(name="x", bufs=2))
        b_pool = ctx.enter_context(tc.tile_pool(name="b", bufs=1))
        o_pool = ctx.enter_context(tc.tile_pool(name="o", bufs=8))
        ps_pool = ctx.enter_context(tc.tile_pool(name="ps", bufs=8, space="PSUM"))

        bias_sb = b_pool.tile([P, OT], mybir.dt.float32, name="bias_sb", tag="bias_sb")
        nc.sync.dma_start(bias_sb[:], bias[:])

        xh = chunks[0][1]  # first-chunk boundary: split x fills there

        def issue_fills():
            w_tiles, x_tiles = [], []
            for k in range(KT):
                wt = w_pool.tile([P, D], BF16, name=f"wt{k}", tag=f"wt{k}")
                nc.sync.dma_start(wt[:, 0 : D // 2], WT[k * P : (k + 1) * P, 0 : D // 2])
                nc.sync.dma_start(wt[:, D // 2 : D], WT[k * P : (k + 1) * P, D // 2 : D])
                xt = x_pool.tile([P, n_cap], BF16, name=f"xt{k}", tag=f"xt{k}")
                nc.gpsimd.dma_start(xt[:, 0:xh], xT[k * P : (k + 1) * P, 0:xh])
                nc.gpsimd.dma_start(xt[:, xh:n_cap], xT[k * P : (k + 1) * P, xh:n_cap])
                w_tiles.append(wt)
                x_tiles.append(xt)
            return w_tiles, x_tiles

        cur = issue_fills()
        for _rep in range(reps):
            w_tiles, x_tiles = cur
            nxt = issue_fills() if _rep + 1 < reps else None

            for j in range(OT):
                pss = []
                for ci, (cs, cw) in enumerate(chunks):
                    ps = ps_pool.tile([P, max(cw for _, cw in chunks)], mybir.dt.float32, name="ps", tag="ps")
                    pss.append(ps)
                for k in range(KT):
                    for ci, (cs, cw) in enumerate(chunks):
                        nc.tensor.matmul(
                            pss[ci][:, 0:cw],
                            w_tiles[k][:, j * P : (j + 1) * P],
                            x_tiles[k][:, cs : cs + cw],
                            start=(k == 0),
                            stop=(k == KT - 1),
                            skip_group_check=True,
                        )
                for ci, (cs, cw) in enumerate(chunks):
                    o_sb = o_pool.tile([P, max(cw for _, cw in chunks)], BF16, name="o", tag="o")
                    nc.vector.tensor_scalar_add(
                        o_sb[:, 0:cw], pss[ci][:, 0:cw], bias_sb[:, j : j + 1]
                    )
                    out_eng = (nc.scalar, nc.sync, nc.gpsimd)[(j * len(chunks) + ci) % 3]
                    out_eng.dma_start(
                        outT[j * P : (j + 1) * P, cs : cs + cw], o_sb[:, 0:cw]
                    )
            cur = nxt

    nc.compile()
    _check_noload_pairs(nc)
    return nc


def _check_noload_pairs(nc):
    """Every ldweights=False matmul must follow (in PE stream order) either
    an InstLdweights or a matmul with the identical stationary AP —
    otherwise the PE array would hold the wrong weights. Scheduling is
    deterministic at build time, so passing here guarantees correctness on
    device."""
    prev_stationary = None
    for fn in nc.m.functions:
        for blk in fn.blocks:
            for inst in blk.instructions:
                tn = type(inst).__name__
                if tn == "InstLdweights":
                    prev_stationary = str(inst.ins[0])
                elif tn == "InstMatmult":
                    if inst.ldweights is False:
                        assert prev_stationary is not None, (
                            "no-load matmul with no predecessor"
                        )
                        assert prev_stationary == str(inst.ins[1]), (
                            f"no-load matmul stationary mismatch:\n"
                            f"prev: {prev_stationary}\nthis: {inst.ins[1]}"
                        )
                    prev_stationary = str(inst.ins[1])


class _Runner:
    """Caches the compiled NEFF + jitted shard_map executable for one n_cap."""

    def __init__(self, n_cap: int, reps: int = 1):
        self.n_cap = n_cap
        self.nc = _build(n_cap, reps)
        _b2j.install_neuronx_cc_hook()

        assert self.nc.dbg_addr is None
        partition_name = (
            self.nc.partition_id_tensor.name if self.nc.partition_id_tensor else None
        )

        in_names, out_names, out_avals = [], [], []
        for alloc in self.nc.m.functions[0].allocations:
            if not isinstance(alloc, mybir.MemoryLocationSet):
                continue
            name = alloc.memorylocations[0].name
            if alloc.kind == "ExternalInput":
                if name != partition_name:
                    in_names.append(name)
            elif alloc.kind == "ExternalOutput":
                out_names.append(name)
                out_avals.append(
                    jax.core.ShapedArray(
                        tuple(alloc.tensor_shape), mybir.dt.np(alloc.dtype)
                    )
                )
        self.in_names = in_names
        self.out_names = out_names
        self.out_avals = out_avals
        self.n_params = len(in_names)
        self.n_outs = len(out_names)
        all_in_names = tuple(in_names + out_names)
        if partition_name is not None:
            all_in_names = all_in_names + (partition_name,)

        nc = self.nc

        def _bind(*args):
            operands = list(args)
            if partition_name is not None:
                operands.append(_b2j.partition_id_tensor())
            return tuple(
                _b2j._bass_exec_p.bind(
                    *operands,
                    out_avals=tuple(out_avals),
                    in_names=all_in_names,
                    out_names=tuple(out_names),
                    lowering_input_output_aliases=(),
                    sim_require_finite=True,
                    sim_require_nnan=True,
                    nc=nc,
                )
            )

        self._bind = _bind
        self.devices = jax.devices("neuron")[:C]
        self.mesh = Mesh(np.asarray(self.devices), ("core",))
        spec_in = (PartitionSpec("core"),) * (self.n_params + self.n_outs)
        spec_out = (PartitionSpec("core"),) * self.n_outs
        self._spec_in, self._spec_out = spec_in, spec_out
        self._exec = jax.jit(
            shard_map(
                _bind,
                mesh=self.mesh,
                in_specs=spec_in,
                out_specs=spec_out,
                check_rep=False,
            ),
            donate_argnums=tuple(range(self.n_params, self.n_params + self.n_outs)),
            keep_unused=True,
        )

    def make_exec_nodonate(self):
        """Jitted executable that does not donate its output-init operands,
        so pre-staged device args can be reused across timing reps."""
        return jax.jit(
            shard_map(
                self._bind,
                mesh=self.mesh,
                in_specs=self._spec_in,
                out_specs=self._spec_out,
                check_rep=False,
            ),
            keep_unused=True,
        )

    def concat_inputs(self, in_maps):
        return [
            np.concatenate([np.asarray(m[name]) for m in in_maps], axis=0)
            for name in self.in_names
        ]

    def zero_outs(self):
        return [
            np.zeros((C * a.shape[0], *a.shape[1:]), a.dtype) for a in self.out_avals
        ]

    def run(self, in_maps):
        out_arrs = self._exec(*self.concat_inputs(in_maps), *self.zero_outs())
        return [
            {
                name: np.asarray(out_arrs[i]).reshape(C, *self.out_avals[i].shape)[c]
                for i, name in enumerate(self.out_names)
            }
            for c in range(C)
        ]


def _get(n_cap: int, reps: int = 1) -> _Runner:
    key = (n_cap, reps)
    if key not in _cache:
        _cache[key] = _Runner(n_cap, reps)
    return _cache[key]


def _prep(x, condition_ids, W, b):
    x = np.ascontiguousarray(np.asarray(x, dtype=np.float32))
    cond = np.asarray(condition_ids).astype(np.int64)
    W = np.asarray(W, dtype=np.float32)
    b = np.asarray(b, dtype=np.float32)

    bias_vec = b.sum(axis=0, dtype=np.float32)  # [D]
    bias_tile = np.ascontiguousarray(bias_vec.reshape(OT, P).T)  # [P, OT]

    rows = [np.nonzero(cond == c)[0] for c in range(C)]
    n_max = max(len(r) for r in rows)
    n_cap = max(32, -(-n_max // 16) * 16)

    in_maps = []
    for c in range(C):
        r = rows[c]
        xg = np.zeros((n_cap, D), np.float32)
        xg[: len(r)] = x[r]
        in_maps.append(
            {
                "xT": np.ascontiguousarray(xg.T.astype(NP_BF16)),
                "WT": np.ascontiguousarray(W[c].T.astype(NP_BF16)),
                "bias": bias_tile,
            }
        )
    return rows, n_cap, in_maps


def _run(x, condition_ids, W, b, trace=False):
    rows, n_cap, in_maps = _prep(x, condition_ids, W, b)
    runner = _get(n_cap)
    results = runner.run(in_maps)

    out = np.empty((np.asarray(x).shape[0], D), np.float32)
    for c in range(C):
        r = rows[c]
        out[r] = results[c]["outT"].T[: len(r)].astype(np.float32)
    return out, runner


def kernel(x, condition_ids, W, b):
    out, _ = _run(x, condition_ids, W, b)
    return out
